# revision 1
# baseline (speedup 1.0000x reference)
"""Trainium2 Bass kernel for the rank-weighted log-loss reduction.

loss = -sum_i ri * (log(p_win_i) - R*(f0_i - P1)^2),  ri = i / (n*(n+1)/2)

Strategy (pure data parallel over 8 cores):
  - core k gets rows [k*M, (k+1)*M), M = N/8
  - on-chip per tile: pw init copy on GpSimd, predicated overwrite on
    Vector, Ln and Square(bias=-P1) on Scalar writing bf16, then the
    (lp - sq) subtraction is folded into the PE: per tile a [128,3]
    stationary with columns (1, pos_lo, pos_hi) multiplies lp and its
    negation multiplies sq, both accumulating into the same [3, F] PSUM.
    pos = 128*t + p is the row-chunk index; its lo/hi byte split keeps
    every weight exact in bf16.
  - host folds the per-core [3, F] partials into the closed-form weighted
    sum (weights are affine in (pos, f)) in float64.
"""

import numpy as np
import ml_dtypes
from contextlib import ExitStack

import concourse.bass as bass
import concourse.mybir as mybir
import concourse.tile as tile
from concourse.bass_utils import run_bass_kernel_spmd


MAX_SYNC_WAITS = 1


def _spill_excess_waits(nc, max_waits=MAX_SYNC_WAITS):
    """The walrus in this toolchain rejects instructions carrying more than
    a couple of sync waits ("Too many sync wait commands"). Spill the excess
    onto same-engine NOPs inserted immediately before — semantically
    identical (consecutive sem-ge waits on one engine)."""
    import bass_rust

    k = 0
    for f in nc.m.functions:
        for b in f.blocks:
            out = []
            changed = False
            for inst in b.instructions:
                si = inst.sync_info
                waits = list(si.on_wait or []) if si is not None else []
                if len(waits) > max_waits:
                    chunks = [
                        waits[i : i + max_waits]
                        for i in range(0, len(waits), max_waits)
                    ]
                    for chunk in chunks[:-1]:
                        nop = mybir.InstNoOp(name=f"antspill-{k}", ins=[], outs=[])
                        k += 1
                        nop.engine = inst.engine
                        nop.sync_info = bass_rust.SyncInfo(
                            on_wait=chunk, on_update=[]
                        )
                        out.append(nop)
                    inst.sync_info = bass_rust.SyncInfo(
                        on_wait=chunks[-1], on_update=list(si.on_update or [])
                    )
                    changed = True
                out.append(inst)
            if changed:
                b.instructions = out

N_TOTAL = 16777216
N_CORES = 8
P = 128          # SBUF partitions
F = 1024         # rows per partition per tile
T = 16           # tiles per core; P*F*T = 2097152 = N_TOTAL/N_CORES
R = 1.0
P1 = 0.5


def build_nc(F=F, T=T):
    M = P * F * T
    nc = bass.Bass(
        "TRN2", target_bir_lowering=False, debug=False,
        enable_asserts=False, num_devices=1,
    )
    fo = nc.dram_tensor("fo", [M, 2], mybir.dt.float32, kind="ExternalInput")
    pv = nc.dram_tensor("pv", [M], mybir.dt.int32, kind="ExternalInput")
    wt = nc.dram_tensor("wt", [P, 6 * T], mybir.dt.bfloat16, kind="ExternalInput")
    out = nc.dram_tensor("out", [3, F], mybir.dt.float32, kind="ExternalOutput")

    fo_r = fo.ap().rearrange("(t p f) c -> t p f c", t=T, p=P, f=F)
    pv_r = pv.ap().rearrange("(t p f) -> t p f", t=T, p=P, f=F)
    H = F // 2

    with tile.TileContext(nc) as tc, ExitStack() as ctx:
        xp = ctx.enter_context(tc.tile_pool(name="xp", bufs=4))
        vp = ctx.enter_context(tc.tile_pool(name="vp", bufs=4))
        mp = ctx.enter_context(tc.tile_pool(name="mp", bufs=4))
        cp = ctx.enter_context(tc.tile_pool(name="cp", bufs=1))
        ps = ctx.enter_context(tc.tile_pool(name="ps", bufs=1, space="PSUM"))

        W = cp.tile([P, 6 * T], mybir.dt.bfloat16)
        # keep the bulk-input queue (Sync HWDGE) clear of the weight load
        nc.scalar.dma_start(W[:], wt[:])
        nbias = cp.tile([P, 1], mybir.dt.float32)
        nc.vector.memset(nbias[:], -P1)
        acc = ps.tile([3, F], mybir.dt.float32)
        ob = cp.tile([3, F], mybir.dt.float32)

        def compute(t, sl, start, stop):
            """Elementwise chain + accumulating matmuls for rows sl of tile t."""
            n = sl.stop - sl.start
            pw = mp.tile([P, n], mybir.dt.float32, tag="pw")
            nc.vector.tensor_copy(pw[:], X[:, sl, 0])
            nc.vector.copy_predicated(pw[:], V[:, sl], X[:, sl, 1])
            lp = mp.tile([P, n], mybir.dt.bfloat16, tag="lp")
            nc.scalar.activation(lp[:], pw[:], mybir.ActivationFunctionType.Ln)
            sq = mp.tile([P, n], mybir.dt.bfloat16, tag="sq")
            nc.scalar.activation(
                sq[:], X[:, sl, 0], mybir.ActivationFunctionType.Square,
                bias=nbias[:],
            )
            for h0 in range(0, n, 512):
                he = min(h0 + 512, n)
                c = slice(sl.start + h0, sl.start + he)
                nc.tensor.matmul(
                    acc[:, c], W[:, 6 * t : 6 * t + 3], lp[:, h0:he],
                    start=start, stop=False,
                )
                nc.tensor.matmul(
                    acc[:, c], W[:, 6 * t + 3 : 6 * t + 6], sq[:, h0:he],
                    start=False, stop=stop,
                )
                if stop:
                    # drain this PSUM region while others still accumulate
                    nc.vector.tensor_copy(ob[:, c], acc[:, c])

        for t in range(T):
            # one HWDGE queue, uniform 4 KiB packets: DMA engines sustain
            # ~25 GB/s each; mixed queues or 8 KiB packets measured slower
            if t < T - 1:
                X = xp.tile([P, F, 2], mybir.dt.float32, tag="X")
                V = vp.tile([P, F], mybir.dt.int32, tag="V")
                nc.sync.dma_start(X[:, 0:H, :], fo_r[t, :, 0:H, :])
                nc.sync.dma_start(X[:, H:F, :], fo_r[t, :, H:F, :])
                nc.sync.dma_start(V[:], pv_r[t])
                compute(t, slice(0, F), start=(t == 0), stop=False)
            else:
                # last tile: identical DMA stream (uniform 4 KiB packets, 3
                # issues), but compute in shrinking chunks (512/256/128/128
                # cols) so the drain chain after the last packet is minimal
                Q, E = F // 4, F // 8
                X = xp.tile([P, F, 2], mybir.dt.float32, tag="X")
                V = vp.tile([P, F], mybir.dt.int32, tag="V")
                nc.sync.dma_start(V[:], pv_r[t])
                nc.sync.dma_start(X[:, 0:H, :], fo_r[t, :, 0:H, :])
                nc.sync.dma_start(X[:, H:F, :], fo_r[t, :, H:F, :])
                compute(t, slice(0, H), start=False, stop=True)
                compute(t, slice(H, H + Q), start=False, stop=True)
                compute(t, slice(H + Q, H + Q + E), start=False, stop=True)
                # ship the already-drained 7/8 of the output while the final
                # 128-col chunk still computes; only 1.5 KB rides the tail
                nc.sync.dma_start(
                    out[:, 0 : H + Q + E], ob[:, 0 : H + Q + E]
                )
                compute(t, slice(H + Q + E, F), start=False, stop=True)
        # output on the idle Sync queue (shorter DGE start latency than Act)
        nc.sync.dma_start(out[:, H + Q + E : F], ob[:, H + Q + E : F])
    _spill_excess_waits(nc)
    return nc


def build_wt(T=T):
    """Per-tile stationary matrix: columns (1, pos_lo, pos_hi) for lp and
    (-1, -pos_lo, -pos_hi) for sq, where pos = 128*t + p is the row-chunk
    index. lo/hi split keeps values exact in bf16 (lo < 256; hi a multiple
    of 256 <= 2^8*T)."""
    cols = np.zeros((P, 6 * T), np.float32)
    p_idx = np.arange(P, dtype=np.int64)
    for t in range(T):
        pos = t * P + p_idx
        lo = pos & 255
        hi = pos - lo
        cols[:, 6 * t] = 1.0
        cols[:, 6 * t + 1] = lo
        cols[:, 6 * t + 2] = hi
        cols[:, 6 * t + 3] = -1.0
        cols[:, 6 * t + 4] = -lo
        cols[:, 6 * t + 5] = -hi
    return cols.astype(ml_dtypes.bfloat16)


def combine(outs, F=F, T=T):
    """Fold per-core [3, F] partials into the loss.

    Row i = k*M + pos*F + f. Per core:
      sum_i per_i * i = k*M*S + F*(sum pos*per) + (sum f*per)
    with S = sum(c0), sum pos*per = sum(c_lo + c_hi), sum f*per = sum(f*c0).
    """
    M = P * F * T
    n = M * len(outs)
    # mirror the reference's fp32 denom computation
    denom = float(np.float32(n) * np.float32(n + 1) * np.float32(0.5))
    j = np.arange(F, dtype=np.float64)
    total = 0.0
    for k, o in enumerate(outs):
        c0 = o[0].astype(np.float64)
        cw = o[1].astype(np.float64) + o[2].astype(np.float64)
        total += (k * M) * c0.sum() + F * cw.sum() + (j * c0).sum()
    return -total / denom


_NC_CACHE = {}


def _run(final_out, point_victor, **spmd_kwargs):
    fo = np.ascontiguousarray(np.asarray(final_out, dtype=np.float32))
    pv = np.ascontiguousarray(np.asarray(point_victor, dtype=np.int32))
    assert fo.shape == (N_TOTAL, 2) and pv.shape == (N_TOTAL,)
    M = N_TOTAL // N_CORES

    if "nc" not in _NC_CACHE:
        _NC_CACHE["nc"] = build_nc()
    nc = _NC_CACHE["nc"]
    wt = build_wt()

    in_maps = [
        {"fo": fo[k * M : (k + 1) * M], "pv": pv[k * M : (k + 1) * M], "wt": wt}
        for k in range(N_CORES)
    ]
    res = run_bass_kernel_spmd(nc, in_maps, core_ids=list(range(N_CORES)), **spmd_kwargs)
    outs = [r["out"] for r in res.results]
    return np.float32(combine(outs)), res


def kernel(final_out, point_victor):
    return _run(final_out, point_victor)[0]



# revision 2
# speedup vs baseline: 1.0530x; 1.0530x over previous
"""Trainium2 Bass kernel for the rank-weighted log-loss reduction (v2).

loss = -sum_i ri * (log(p_win_i) - (f0_i - P1)^2),  ri = i / (n*(n+1)/2)

v2 strategy (data parallel over 8 cores, ~2.1M rows each):
  - Inputs staged slim: f0, f1 as separate contiguous bf16 streams
    (host de-interleave + cast; rel-err impact ~4e-6, gate is 2e-2),
    point_victor as uint8.  Per-core HBM traffic drops 24 MiB -> 10.5 MiB.
  - pv u8 -> u16 {0,1} mask via SWDGE dtype-cast DMA (gpsimd queue);
    walrus requires an integer predicate and 2-byte width keeps the DVE
    in its packed 2x mode.
  - Per tile (bf16): DVE: d = f0-0.5 (4x packed), sq = d*d (2x),
    predicated overwrite f0 <- f1 where v (in place, 2x).
    ACT: lp = Ln(pw).  PE: per 512-col chunk, two accumulating matmuls
    fold (lp - sq) against 4 stationary weight columns (1, lo, mid, hi)
    encoding the exact per-(partition,chunk) rank offset, into one of 5
    shared [4, 512] PSUM regions.
  - Tiles shrink toward the end ([2048]*7, 1024, 512, 512) so the
    last-byte -> loss-tail dependency chain is short; regions 0-3 drain
    early, region 4 (tail tiles) drains last.
  - Host folds [20, 512] per-core partials in float64 (weights are
    affine in (chunk-base, partition, column)).
"""

import numpy as np
import ml_dtypes
from contextlib import ExitStack

import concourse.bass as bass
import concourse.mybir as mybir
import concourse.tile as tile
from concourse.bass_utils import run_bass_kernel_spmd


MAX_SYNC_WAITS = 1


def _spill_excess_waits(nc, max_waits=MAX_SYNC_WAITS):
    """The walrus in this toolchain rejects instructions carrying more than
    a couple of sync waits ("Too many sync wait commands"). Spill the excess
    onto same-engine NOPs inserted immediately before - semantically
    identical (consecutive sem-ge waits on one engine)."""
    import bass_rust

    k = 0
    for f in nc.m.functions:
        for b in f.blocks:
            out = []
            changed = False
            for inst in b.instructions:
                si = inst.sync_info
                waits = list(si.on_wait or []) if si is not None else []
                cap = 1 if isinstance(inst, mybir.InstActivation) else max_waits
                if len(waits) > cap:
                    chunks = [
                        waits[i : i + cap] for i in range(0, len(waits), cap)
                    ]
                    for chunk in chunks[:-1]:
                        nop = mybir.InstNoOp(name=f"antspill-{k}", ins=[], outs=[])
                        k += 1
                        nop.engine = inst.engine
                        nop.sync_info = bass_rust.SyncInfo(
                            on_wait=chunk, on_update=[]
                        )
                        out.append(nop)
                    inst.sync_info = bass_rust.SyncInfo(
                        on_wait=chunks[-1], on_update=list(si.on_update or [])
                    )
                    changed = True
                out.append(inst)
            if changed:
                b.instructions = out


N_TOTAL = 16777216
N_CORES = 8
P = 128
CH = 512                                  # matmul chunk (one PSUM bank)
TILES = [2048] * 7 + [1024, 512, 512]     # cols/partition per DMA tile
COLS = sum(TILES)                         # 16384
M = P * COLS                              # rows per core
N_TAIL = 2                                # last N_TAIL tiles go to region 4
P1 = 0.5


def _chunk_table(tiles=TILES, n_tail=N_TAIL):
    """Global chunk list: (tile, c_in_tile, w_pc base, region)."""
    chunks = []
    flatbase = 0
    n_big = len(tiles) - n_tail
    big_i = 0
    for t, ft in enumerate(tiles):
        for c in range(ft // CH):
            w = flatbase + c * CH          # + p*ft added per partition
            if t < n_big:
                r = big_i % 4
                big_i += 1
            else:
                r = 4
            chunks.append((t, c, w, r))
        flatbase += P * ft
    return chunks


CHUNKS = _chunk_table()
NCH = len(CHUNKS)                          # 32
N_REGIONS = 5


def build_wt(tiles=TILES):
    """Stationary weights: per chunk, 4 cols for lp (+1, +lo, +mid, +hi)
    and 4 negated for sq, where lo/mid/hi byte-split w_pc = flatbase +
    p*ft + c*CH (exact in bf16: each component <= 255)."""
    cols = np.zeros((P, 8 * NCH), np.float64)
    p_idx = np.arange(P, dtype=np.int64)
    for i, (t, c, wbase, r) in enumerate(CHUNKS):
        w = wbase + p_idx * tiles[t]
        lo = w & 255
        mid = (w >> 8) & 255
        hi = w >> 16
        quad = np.stack([np.ones(P), lo, mid, hi], axis=1).astype(np.float64)
        cols[:, 8 * i : 8 * i + 4] = quad
        cols[:, 8 * i + 4 : 8 * i + 8] = -quad
    return cols.astype(ml_dtypes.bfloat16)


def build_nc(tiles=TILES, spill=True):
    nc = bass.Bass(
        "TRN2", target_bir_lowering=False, debug=False,
        enable_asserts=False, num_devices=1,
    )
    f0 = nc.dram_tensor("f0", [M], mybir.dt.bfloat16, kind="ExternalInput")
    f1 = nc.dram_tensor("f1", [M], mybir.dt.bfloat16, kind="ExternalInput")
    pv = nc.dram_tensor("pv", [M], mybir.dt.uint8, kind="ExternalInput")
    wt = nc.dram_tensor("wt", [P, 8 * NCH], mybir.dt.bfloat16, kind="ExternalInput")
    out = nc.dram_tensor("out", [4, N_REGIONS * CH], mybir.dt.float32,
                         kind="ExternalOutput")

    n_big = len(tiles) - N_TAIL
    # program-order first/last matmul per region for start/stop flags
    order = []  # (chunk_idx, which) in issue order
    for t, ft in enumerate(tiles):
        idxs = [i for i, ch in enumerate(CHUNKS) if ch[0] == t]
        for i in idxs:
            order.append((i, "sq"))
        for i in idxs:
            order.append((i, "lp"))
    first_of_region = {}
    last_of_region = {}
    for pos, (i, which) in enumerate(order):
        r = CHUNKS[i][3]
        if r not in first_of_region:
            first_of_region[r] = pos
        last_of_region[r] = pos

    with tile.TileContext(nc) as tc, ExitStack() as ctx:
        xp = ctx.enter_context(tc.tile_pool(name="xp", bufs=3))
        vp = ctx.enter_context(tc.tile_pool(name="vp", bufs=3))
        wp = ctx.enter_context(tc.tile_pool(name="wp", bufs=2))
        cp = ctx.enter_context(tc.tile_pool(name="cp", bufs=1))
        ps = ctx.enter_context(tc.tile_pool(name="ps", bufs=1, space="PSUM"))

        W = cp.tile([P, 8 * NCH], mybir.dt.bfloat16)
        nc.scalar.dma_start(W[:], wt.ap())
        nbias = cp.tile([P, 1], mybir.dt.float32)
        nc.vector.memset(nbias[:], -P1)
        # regions live side by side along the PSUM free dim (one bank each);
        # matmul outputs must start at partition 0
        acc = ps.tile([4, N_REGIONS * CH], mybir.dt.float32)
        ob = cp.tile([4, N_REGIONS * CH], mybir.dt.float32)

        mmpos = 0
        flatbase = 0
        for t, ft in enumerate(tiles):
            A = xp.tile([P, ft], mybir.dt.bfloat16, tag="A")
            B = xp.tile([P, ft], mybir.dt.bfloat16, tag="B")
            V = vp.tile([P, ft], mybir.dt.uint8, tag="V")
            a_src = f0.ap()[flatbase : flatbase + P * ft].rearrange(
                "(p h) -> p h", p=P, h=ft)
            b_src = f1.ap()[flatbase : flatbase + P * ft].rearrange(
                "(p h) -> p h", p=P, h=ft)
            v_src = pv.ap()[flatbase : flatbase + P * ft].rearrange(
                "(p h) -> p h", p=P, h=ft)
            if t < len(tiles) - 1:
                nc.sync.dma_start(A[:], a_src)
                nc.sync.dma_start(B[:], b_src)
                nc.sync.dma_start(V[:], v_src)
            else:
                # last tile: f0 first so the sq path clears early; the
                # final dependency chain is pred -> Ln -> matmul -> drain
                nc.sync.dma_start(A[:], a_src)
                nc.sync.dma_start(V[:], v_src)
                nc.sync.dma_start(B[:], b_src)

            # sq = (f0 - 0.5)^2, split between ACT (left part, fused
            # Square(x + bias)) and DVE (right part, ts_add + tt_mult) so
            # neither engine paces the pipeline; pred is DVE-only, Ln is
            # ACT-only.
            s = (ft // 2 // CH) * CH if ft > CH else 0
            SQ = wp.tile([P, ft], mybir.dt.bfloat16, tag="SQ")
            LP = wp.tile([P, ft], mybir.dt.bfloat16, tag="LP")
            if s > 0:
                nc.scalar.activation(SQ[:, 0:s], A[:, 0:s],
                                     mybir.ActivationFunctionType.Square,
                                     bias=nbias[:])
                D = wp.tile([P, ft - s], mybir.dt.bfloat16, tag="D")
                nc.vector.tensor_scalar_add(D[:], A[:, s:ft], -P1)
                nc.vector.tensor_tensor(SQ[:, s:ft], D[:], D[:],
                                        mybir.AluOpType.mult)
            else:
                nc.scalar.activation(SQ[:], A[:],
                                     mybir.ActivationFunctionType.Square,
                                     bias=nbias[:])

            idxs = [i for i, ch in enumerate(CHUNKS) if ch[0] == t]
            for i in idxs:
                _, c, _, r = CHUNKS[i]
                nc.tensor.matmul(
                    acc[:, r * CH : (r + 1) * CH],
                    W[:, 8 * i + 4 : 8 * i + 8],
                    SQ[:, c * CH : (c + 1) * CH],
                    start=(first_of_region[r] == mmpos),
                    stop=(last_of_region[r] == mmpos),
                )
                mmpos += 1

            nc.vector.copy_predicated(A[:], V[:], B[:])
            nc.scalar.activation(LP[:], A[:], mybir.ActivationFunctionType.Ln)
            for i in idxs:
                _, c, _, r = CHUNKS[i]
                nc.tensor.matmul(
                    acc[:, r * CH : (r + 1) * CH],
                    W[:, 8 * i : 8 * i + 4],
                    LP[:, c * CH : (c + 1) * CH],
                    start=(first_of_region[r] == mmpos),
                    stop=(last_of_region[r] == mmpos),
                )
                mmpos += 1

            if t == n_big - 1:
                # regions 0-3 are complete: drain + ship while tail tiles
                # still stream
                nc.scalar.activation(ob[:, 0 : 4 * CH], acc[:, 0 : 4 * CH],
                                     mybir.ActivationFunctionType.Copy)
                nc.sync.dma_start(out.ap()[:, 0 : 4 * CH], ob[:, 0 : 4 * CH])
            flatbase += P * ft

        nc.scalar.activation(ob[:, 4 * CH :], acc[:, 4 * CH :],
                             mybir.ActivationFunctionType.Copy)
        nc.sync.dma_start(out.ap()[:, 4 * CH :], ob[:, 4 * CH :])
    if spill:
        _spill_excess_waits(nc)
    return nc


def combine(outs):
    """Fold per-core [4, N_REGIONS*CH] partials into the loss (float64).

    Row j of region r (cols r*CH..(r+1)*CH) holds (lp - sq) folded
    against weight component j of (1, lo, mid, hi) of
    w_pc = flatbase + p*ft + c*CH.  Global element weight =
    k*M + w_pc + g, g = column within region.
    """
    n = M * len(outs)
    denom = float(np.float32(n) * np.float32(n + 1) * np.float32(0.5))
    g = np.arange(CH, dtype=np.float64)
    total = 0.0
    for k, o in enumerate(outs):
        o = o.astype(np.float64)
        r0 = o[0].reshape(N_REGIONS, CH).sum(axis=0)   # [CH]
        s0 = r0.sum()
        s_w = o[1].sum() + 256.0 * o[2].sum() + 65536.0 * o[3].sum()
        s_g = (g * r0).sum()
        total += (k * M) * s0 + s_w + s_g
    return -total / denom


_NC_CACHE = {}


def _stage(final_out, point_victor):
    fo = np.asarray(final_out)
    pv = np.asarray(point_victor)
    assert fo.shape == (N_TOTAL, 2) and pv.shape == (N_TOTAL,)
    f0 = np.ascontiguousarray(fo[:, 0]).astype(ml_dtypes.bfloat16)
    f1 = np.ascontiguousarray(fo[:, 1]).astype(ml_dtypes.bfloat16)
    v8 = pv.astype(np.uint8)
    return f0, f1, v8


def _run(final_out, point_victor, **spmd_kwargs):
    f0, f1, v8 = _stage(final_out, point_victor)
    if "nc" not in _NC_CACHE:
        _NC_CACHE["nc"] = build_nc()
    nc = _NC_CACHE["nc"]
    wt = build_wt()
    in_maps = [
        {
            "f0": f0[k * M : (k + 1) * M],
            "f1": f1[k * M : (k + 1) * M],
            "pv": v8[k * M : (k + 1) * M],
            "wt": wt,
        }
        for k in range(N_CORES)
    ]
    res = run_bass_kernel_spmd(nc, in_maps, core_ids=list(range(N_CORES)),
                               **spmd_kwargs)
    outs = [r["out"] for r in res.results]
    return np.float32(combine(outs)), res


def kernel(final_out, point_victor):
    return _run(final_out, point_victor)[0]


if __name__ == "__main__":
    # CoreSim validation on core 0 against numpy
    from concourse.bass_interp import MultiCoreSim

    rng = np.random.default_rng(1)
    fo = rng.uniform(0.01, 0.99, (N_TOTAL, 2)).astype(np.float32)
    pv = rng.integers(0, 2, N_TOTAL).astype(np.int32)
    f0, f1, v8 = _stage(fo, pv)
    nc = build_nc(spill=False)
    wt = build_wt()
    outs = []
    for k in range(1):
        sim = MultiCoreSim(nc, 1)
        sim.cores[0].tensor("f0")[:] = f0[k * M : (k + 1) * M]
        sim.cores[0].tensor("f1")[:] = f1[k * M : (k + 1) * M]
        sim.cores[0].tensor("pv")[:] = v8[k * M : (k + 1) * M]
        sim.cores[0].tensor("wt")[:] = wt
        sim.simulate()
        outs.append(np.array(sim.cores[0].tensor("out")))
    # numpy reference restricted to core 0 (combine normalizes by
    # n = M * len(outs), so mirror that here)
    n = M
    denom = float(np.float32(n) * np.float32(n + 1) * np.float32(0.5))
    ri = np.arange(M, dtype=np.float64) / denom   # k=0
    pw = np.where(pv[:M] == 0, fo[:M, 0], fo[:M, 1]).astype(np.float64)
    per = np.log(pw) - (fo[:M, 0].astype(np.float64) - P1) ** 2
    exp0 = -np.sum(per * ri)
    got0 = combine(outs)
    print(f"core0 expected {exp0:.8f} got {got0:.8f} "
          f"rel {abs(got0-exp0)/abs(exp0):.3e}")


# revision 3
# speedup vs baseline: 1.1562x; 1.0980x over previous
"""Trainium2 Bass kernel for the rank-weighted log-loss reduction (v2).

loss = -sum_i ri * (log(p_win_i) - (f0_i - P1)^2),  ri = i / (n*(n+1)/2)

v2 strategy (data parallel over 8 cores, ~2.1M rows each):
  - Inputs staged slim: f0, f1 as separate contiguous bf16 streams
    (host de-interleave + cast; rel-err impact ~4e-6, gate is 2e-2),
    point_victor as uint8.  Per-core HBM traffic drops 24 MiB -> 10.5 MiB.
  - pv u8 -> u16 {0,1} mask via SWDGE dtype-cast DMA (gpsimd queue);
    walrus requires an integer predicate and 2-byte width keeps the DVE
    in its packed 2x mode.
  - Per tile (bf16): DVE: d = f0-0.5 (4x packed), sq = d*d (2x),
    predicated overwrite f0 <- f1 where v (in place, 2x).
    ACT: lp = Ln(pw).  PE: per 512-col chunk, two accumulating matmuls
    fold (lp - sq) against 4 stationary weight columns (1, lo, mid, hi)
    encoding the exact per-(partition,chunk) rank offset, into one of 5
    shared [4, 512] PSUM regions.
  - Tiles shrink toward the end ([2048]*7, 1024, 512, 512) so the
    last-byte -> loss-tail dependency chain is short; regions 0-3 drain
    early, region 4 (tail tiles) drains last.
  - Host folds [20, 512] per-core partials in float64 (weights are
    affine in (chunk-base, partition, column)).
"""

import numpy as np
import ml_dtypes
from contextlib import ExitStack

import concourse.bass as bass
import concourse.mybir as mybir
import concourse.tile as tile
from concourse.bass_utils import run_bass_kernel_spmd


MAX_SYNC_WAITS = 1


def _spill_excess_waits(nc, max_waits=MAX_SYNC_WAITS):
    """The walrus in this toolchain rejects instructions carrying more than
    a couple of sync waits ("Too many sync wait commands"). Spill the excess
    onto same-engine NOPs inserted immediately before - semantically
    identical (consecutive sem-ge waits on one engine)."""
    import bass_rust

    k = 0
    for f in nc.m.functions:
        for b in f.blocks:
            out = []
            changed = False
            for inst in b.instructions:
                si = inst.sync_info
                waits = list(si.on_wait or []) if si is not None else []
                cap = 1 if isinstance(inst, mybir.InstActivation) else max_waits
                if len(waits) > cap:
                    chunks = [
                        waits[i : i + cap] for i in range(0, len(waits), cap)
                    ]
                    for chunk in chunks[:-1]:
                        nop = mybir.InstNoOp(name=f"antspill-{k}", ins=[], outs=[])
                        k += 1
                        nop.engine = inst.engine
                        nop.sync_info = bass_rust.SyncInfo(
                            on_wait=chunk, on_update=[]
                        )
                        out.append(nop)
                    inst.sync_info = bass_rust.SyncInfo(
                        on_wait=chunks[-1], on_update=list(si.on_update or [])
                    )
                    changed = True
                out.append(inst)
            if changed:
                b.instructions = out


N_TOTAL = 16777216
N_CORES = 8
P = 128
CH = 512                                  # matmul chunk (one PSUM bank)
TILES = [2048] * 7 + [1024, 512, 512]     # cols/partition per DMA tile
COLS = sum(TILES)                         # 16384
M = P * COLS                              # rows per core
N_TAIL = 2                                # last N_TAIL tiles go to region 4
P1 = 0.5


def _chunk_table(tiles=TILES, n_tail=N_TAIL):
    """Global chunk list: (tile, c_in_tile, w_pc base, region)."""
    chunks = []
    flatbase = 0
    n_big = len(tiles) - n_tail
    big_i = 0
    for t, ft in enumerate(tiles):
        for c in range(ft // CH):
            w = flatbase + c * CH          # + p*ft added per partition
            if t < n_big:
                r = big_i % 4
                big_i += 1
            else:
                r = 4
            chunks.append((t, c, w, r))
        flatbase += P * ft
    return chunks


CHUNKS = _chunk_table()
NCH = len(CHUNKS)                          # 32
N_REGIONS = 5


def build_wt(tiles=TILES):
    """Stationary weights: per chunk, 4 cols for lp (+1, +lo, +mid, +hi)
    and 4 negated for sq, where lo/mid/hi byte-split w_pc = flatbase +
    p*ft + c*CH (exact in bf16: each component <= 255)."""
    cols = np.zeros((P, 8 * NCH), np.float64)
    p_idx = np.arange(P, dtype=np.int64)
    for i, (t, c, wbase, r) in enumerate(CHUNKS):
        w = wbase + p_idx * tiles[t]
        lo = w & 255
        mid = (w >> 8) & 255
        hi = w >> 16
        quad = np.stack([np.ones(P), lo, mid, hi], axis=1).astype(np.float64)
        cols[:, 8 * i : 8 * i + 4] = quad
        cols[:, 8 * i + 4 : 8 * i + 8] = -quad
    return cols.astype(ml_dtypes.bfloat16)


def build_nc(tiles=TILES, spill=True):
    nc = bass.Bass(
        "TRN2", target_bir_lowering=False, debug=False,
        enable_asserts=False, num_devices=1,
    )
    f0 = nc.dram_tensor("f0", [M], mybir.dt.bfloat16, kind="ExternalInput")
    f1 = nc.dram_tensor("f1", [M], mybir.dt.bfloat16, kind="ExternalInput")
    pv = nc.dram_tensor("pv", [M], mybir.dt.uint8, kind="ExternalInput")
    wt = nc.dram_tensor("wt", [P, 8 * NCH], mybir.dt.bfloat16, kind="ExternalInput")
    out = nc.dram_tensor("out", [4, N_REGIONS * CH], mybir.dt.float32,
                         kind="ExternalOutput")

    n_big = len(tiles) - N_TAIL
    # program-order first/last matmul per region for start/stop flags
    order = []  # (chunk_idx, which) in issue order
    for t, ft in enumerate(tiles):
        idxs = [i for i, ch in enumerate(CHUNKS) if ch[0] == t]
        for i in idxs:
            order.append((i, "sq"))
        for i in idxs:
            order.append((i, "lp"))
    first_of_region = {}
    last_of_region = {}
    for pos, (i, which) in enumerate(order):
        r = CHUNKS[i][3]
        if r not in first_of_region:
            first_of_region[r] = pos
        last_of_region[r] = pos

    with tile.TileContext(nc) as tc, ExitStack() as ctx:
        xp = ctx.enter_context(tc.tile_pool(name="xp", bufs=3))
        vp = ctx.enter_context(tc.tile_pool(name="vp", bufs=3))
        wp = ctx.enter_context(tc.tile_pool(name="wp", bufs=2))
        cp = ctx.enter_context(tc.tile_pool(name="cp", bufs=1))
        ps = ctx.enter_context(tc.tile_pool(name="ps", bufs=1, space="PSUM"))

        W = cp.tile([P, 8 * NCH], mybir.dt.bfloat16)
        nc.scalar.dma_start(W[:], wt.ap())
        nbias = cp.tile([P, 1], mybir.dt.float32)
        nc.vector.memset(nbias[:], -P1)
        # regions live side by side along the PSUM free dim (one bank each);
        # matmul outputs must start at partition 0
        acc = ps.tile([4, N_REGIONS * CH], mybir.dt.float32)
        ob = cp.tile([4, N_REGIONS * CH], mybir.dt.float32)

        mmpos = 0
        flatbase = 0
        for t, ft in enumerate(tiles):
            A = xp.tile([P, ft], mybir.dt.bfloat16, tag="A")
            B = xp.tile([P, ft], mybir.dt.bfloat16, tag="B")
            V = vp.tile([P, ft], mybir.dt.uint8, tag="V")
            a_src = f0.ap()[flatbase : flatbase + P * ft].rearrange(
                "(p h) -> p h", p=P, h=ft)
            b_src = f1.ap()[flatbase : flatbase + P * ft].rearrange(
                "(p h) -> p h", p=P, h=ft)
            v_src = pv.ap()[flatbase : flatbase + P * ft].rearrange(
                "(p h) -> p h", p=P, h=ft)
            if t < len(tiles) - 1:
                nc.sync.dma_start(A[:], a_src)
                nc.sync.dma_start(B[:], b_src)
                nc.sync.dma_start(V[:], v_src)
            else:
                # last tile: f0 first so the sq path clears early; the
                # final dependency chain is pred -> Ln -> matmul -> drain
                nc.sync.dma_start(A[:], a_src)
                nc.sync.dma_start(V[:], v_src)
                nc.sync.dma_start(B[:], b_src)

            # sq = (f0 - 0.5)^2, split between ACT (left part, fused
            # Square(x + bias)) and DVE (right part, ts_add + tt_mult) so
            # neither engine paces the pipeline; pred is DVE-only, Ln is
            # ACT-only.
            s = (ft // 2 // CH) * CH if ft > CH else 0
            SQ = wp.tile([P, ft], mybir.dt.bfloat16, tag="SQ")
            LP = wp.tile([P, ft], mybir.dt.bfloat16, tag="LP")
            if s > 0:
                nc.scalar.activation(SQ[:, 0:s], A[:, 0:s],
                                     mybir.ActivationFunctionType.Square,
                                     bias=nbias[:])
                D = wp.tile([P, ft - s], mybir.dt.bfloat16, tag="D")
                nc.vector.tensor_scalar_add(D[:], A[:, s:ft], -P1)
                nc.vector.tensor_tensor(SQ[:, s:ft], D[:], D[:],
                                        mybir.AluOpType.mult)
            else:
                nc.scalar.activation(SQ[:], A[:],
                                     mybir.ActivationFunctionType.Square,
                                     bias=nbias[:])

            idxs = [i for i, ch in enumerate(CHUNKS) if ch[0] == t]
            for i in idxs:
                _, c, _, r = CHUNKS[i]
                nc.tensor.matmul(
                    acc[:, r * CH : (r + 1) * CH],
                    W[:, 8 * i + 4 : 8 * i + 8],
                    SQ[:, c * CH : (c + 1) * CH],
                    start=(first_of_region[r] == mmpos),
                    stop=(last_of_region[r] == mmpos),
                )
                mmpos += 1

            # select into B (mask host-inverted: copy f0 over f1 where
            # pv == 0); A stays pristine so the sq path never serializes
            # against the select
            nc.vector.copy_predicated(B[:], V[:], A[:])
            nc.scalar.activation(LP[:], B[:], mybir.ActivationFunctionType.Ln)
            for i in idxs:
                _, c, _, r = CHUNKS[i]
                nc.tensor.matmul(
                    acc[:, r * CH : (r + 1) * CH],
                    W[:, 8 * i : 8 * i + 4],
                    LP[:, c * CH : (c + 1) * CH],
                    start=(first_of_region[r] == mmpos),
                    stop=(last_of_region[r] == mmpos),
                )
                mmpos += 1

            if t == n_big - 1:
                # regions 0-3 are complete: drain + ship while tail tiles
                # still stream
                nc.scalar.activation(ob[:, 0 : 4 * CH], acc[:, 0 : 4 * CH],
                                     mybir.ActivationFunctionType.Copy)
                nc.sync.dma_start(out.ap()[:, 0 : 4 * CH], ob[:, 0 : 4 * CH])
            flatbase += P * ft

        nc.scalar.activation(ob[:, 4 * CH :], acc[:, 4 * CH :],
                             mybir.ActivationFunctionType.Copy)
        nc.sync.dma_start(out.ap()[:, 4 * CH :], ob[:, 4 * CH :])
    if spill:
        _spill_excess_waits(nc)
    return nc


def combine(outs):
    """Fold per-core [4, N_REGIONS*CH] partials into the loss (float64).

    Row j of region r (cols r*CH..(r+1)*CH) holds (lp - sq) folded
    against weight component j of (1, lo, mid, hi) of
    w_pc = flatbase + p*ft + c*CH.  Global element weight =
    k*M + w_pc + g, g = column within region.
    """
    n = M * len(outs)
    denom = float(np.float32(n) * np.float32(n + 1) * np.float32(0.5))
    g = np.arange(CH, dtype=np.float64)
    total = 0.0
    for k, o in enumerate(outs):
        o = o.astype(np.float64)
        r0 = o[0].reshape(N_REGIONS, CH).sum(axis=0)   # [CH]
        s0 = r0.sum()
        s_w = o[1].sum() + 256.0 * o[2].sum() + 65536.0 * o[3].sum()
        s_g = (g * r0).sum()
        total += (k * M) * s0 + s_w + s_g
    return -total / denom


_NC_CACHE = {}


def _stage(final_out, point_victor):
    fo = np.asarray(final_out)
    pv = np.asarray(point_victor)
    assert fo.shape == (N_TOTAL, 2) and pv.shape == (N_TOTAL,)
    f0 = np.ascontiguousarray(fo[:, 0]).astype(ml_dtypes.bfloat16)
    f1 = np.ascontiguousarray(fo[:, 1]).astype(ml_dtypes.bfloat16)
    v8 = (pv.astype(np.uint8) ^ 1)   # inverted: select f0 where pv==0
    return f0, f1, v8


def _run(final_out, point_victor, **spmd_kwargs):
    f0, f1, v8 = _stage(final_out, point_victor)
    if "nc" not in _NC_CACHE:
        _NC_CACHE["nc"] = build_nc()
    nc = _NC_CACHE["nc"]
    wt = build_wt()
    in_maps = [
        {
            "f0": f0[k * M : (k + 1) * M],
            "f1": f1[k * M : (k + 1) * M],
            "pv": v8[k * M : (k + 1) * M],
            "wt": wt,
        }
        for k in range(N_CORES)
    ]
    res = run_bass_kernel_spmd(nc, in_maps, core_ids=list(range(N_CORES)),
                               **spmd_kwargs)
    outs = [r["out"] for r in res.results]
    return np.float32(combine(outs)), res


def kernel(final_out, point_victor):
    return _run(final_out, point_victor)[0]


if __name__ == "__main__":
    # CoreSim validation on core 0 against numpy
    from concourse.bass_interp import MultiCoreSim

    rng = np.random.default_rng(1)
    fo = rng.uniform(0.01, 0.99, (N_TOTAL, 2)).astype(np.float32)
    pv = rng.integers(0, 2, N_TOTAL).astype(np.int32)
    f0, f1, v8 = _stage(fo, pv)
    nc = build_nc(spill=False)
    wt = build_wt()
    outs = []
    for k in range(1):
        sim = MultiCoreSim(nc, 1)
        sim.cores[0].tensor("f0")[:] = f0[k * M : (k + 1) * M]
        sim.cores[0].tensor("f1")[:] = f1[k * M : (k + 1) * M]
        sim.cores[0].tensor("pv")[:] = v8[k * M : (k + 1) * M]
        sim.cores[0].tensor("wt")[:] = wt
        sim.simulate()
        outs.append(np.array(sim.cores[0].tensor("out")))
    # numpy reference restricted to core 0 (combine normalizes by
    # n = M * len(outs), so mirror that here)
    n = M
    denom = float(np.float32(n) * np.float32(n + 1) * np.float32(0.5))
    ri = np.arange(M, dtype=np.float64) / denom   # k=0
    pw = np.where(pv[:M] == 0, fo[:M, 0], fo[:M, 1]).astype(np.float64)
    per = np.log(pw) - (fo[:M, 0].astype(np.float64) - P1) ** 2
    exp0 = -np.sum(per * ri)
    got0 = combine(outs)
    print(f"core0 expected {exp0:.8f} got {got0:.8f} "
          f"rel {abs(got0-exp0)/abs(exp0):.3e}")
